# revision 17
# baseline (speedup 1.0000x reference)
"""A-Connect conv kernel for TRN2, data-parallel over batch on 8 NeuronCores.

Computation (per sample b):
    Z[b] = conv2d(X[b], W * Werr[b], SAME) + bias * Berr[b]; out = relu(Z)

Mapping: batch 32 -> 4 samples per core. Per sample the conv is 9
shifted matmuls accumulated in PSUM. The output is computed in the
zero-padded spatial geometry (64 rows x 66 cols = 4224 = 33 tiles of
128 positions): in that flattened geometry the stationary operand for
tap (dy, dx) is a single stride-1 run of the padded input at offset
q0 + dy*66 + dx, which satisfies the BIR rule that matmul operand APs
have one free dimension. The two junk columns (xp = 0, 65) are sliced
away on the host. PSUM/output tiles are [spatial, F], matching NHWC so
stores are contiguous; the per-sample bias is added on the vector
engine in PSUM and relu+copyout runs on the scalar engine. A burst of
dependency-free warmup matmuls at kernel start releases the PE HAM
clock gate while the first input DMAs are in flight. Inputs are
pre-transposed/padded on host and fed in bf16 (measured rel err vs the
fp32 reference: 2.3e-3).
"""

import numpy as np
import ml_dtypes

B, H, Wd, Cin, F, KH, KW = 32, 64, 64, 128, 256, 3, 3
NCORES = 8
BPC = B // NCORES  # samples per core
HP, WP = H + 2, Wd + 2  # zero-padded spatial
NQ = H * WP  # padded output positions per sample: 64*66 = 4224
MT = NQ // 128  # 33 M-tiles of 128 padded positions
XLEN = 4368  # 1 lead zero + 66*66 flat padded image + tail zeros
# X arrives in overlapping chunks (small first) so matmuls can start as
# soon as a few hundred KB have landed. Chunk k serves M-tiles
# [XCH_MT[k], XCH_MT[k+1]) and covers flat [XCH_LO[k], XCH_HI[k]):
# a tile m reads offsets [128m, 128m+134+128).
XCH_MT = [0, 2, 8, 16, 24, MT]
XCH_LO = [0, 256, 1024, 2048, 3072]
XCH_HI = [390, 1158, 2182, 3206, XLEN]

_compiled = None  # cached Bass program so repeated kernel() calls reuse it


def _build_bass():
    from concourse import bacc, tile, mybir

    nc = bacc.Bacc("TRN2", target_bir_lowering=False, debug=False)
    bf16 = mybir.dt.bfloat16
    f32 = mybir.dt.float32

    xp = nc.dram_tensor("xp", [BPC, Cin, XLEN], bf16, kind="ExternalInput")
    wm = nc.dram_tensor("wm", [BPC, Cin, KH * KW, F], bf16, kind="ExternalInput")
    mb = nc.dram_tensor("mb", [BPC, 128, F], f32, kind="ExternalInput")
    y = nc.dram_tensor("y", [BPC, MT, 128, F], f32, kind="ExternalOutput")

    with tile.TileContext(nc) as tc:
        with (
            tc.tile_pool(name="xpool", bufs=2) as xpool,
            tc.tile_pool(name="wpool", bufs=2) as wpool,
            tc.tile_pool(name="bpool", bufs=2) as bpool,
            tc.tile_pool(name="opool", bufs=8) as opool,
            tc.tile_pool(name="cpool", bufs=1) as cpool,
            tc.tile_pool(name="pspool", bufs=7, space="PSUM") as pspool,
            tc.tile_pool(name="wupool", bufs=1, space="PSUM") as wupool,
        ):
            # PE warmup: ~5us of dependency-free matmuls so the HAM clock
            # gate is released (K=8/8) by the time the first input DMA lands
            # 9 cold N=512 matmuls ~= 3.8us: just enough sustained PE
            # activity to release the HAM clock gate, without queueing the
            # first real matmul (PE is FIFO) behind excess warmup work
            wu_in = cpool.tile([128, 512], bf16)
            nc.vector.memset(wu_in[:], 0.0)
            wu_ps = wupool.tile([128, 512], f32)
            for i in range(9):
                nc.tensor.matmul(
                    wu_ps[:],
                    wu_in[:, :128],
                    wu_in[:],
                    start=(i == 0),
                    stop=(i == 8),
                )
            for b in range(BPC):
                # input DMAs ride the scalar engine's HWDGE ring so their
                # issue cost doesn't contend with output DMAs on nc.sync;
                # weights and X stream in small-first chunks to cut the
                # lead-in before the first matmul
                wtc = []
                xcs = []
                for j in range(KH):
                    w_ = wpool.tile([Cin, KW, F], bf16, tag=f"wt{j}")
                    nc.scalar.dma_start(w_[:], wm[b, :, 3 * j : 3 * j + 3, :])
                    wtc.append(w_)
                    if j < len(XCH_LO):
                        lo, hi = XCH_LO[j], XCH_HI[j]
                        xc = xpool.tile([Cin, hi - lo], bf16, tag=f"xc{j}")
                        nc.scalar.dma_start(xc[:], xp[b, :, lo:hi])
                        xcs.append(xc)
                for j in range(KH, len(XCH_LO)):
                    lo, hi = XCH_LO[j], XCH_HI[j]
                    xc = xpool.tile([Cin, hi - lo], bf16, tag=f"xc{j}")
                    nc.scalar.dma_start(xc[:], xp[b, :, lo:hi])
                    xcs.append(xc)
                bt = bpool.tile([128, F], f32)
                nc.scalar.dma_start(bt[:], mb[b])
                ck = 0
                for m in range(MT):
                    q0 = m * 128
                    while m >= XCH_MT[ck + 1]:
                        ck += 1
                    ps = pspool.tile([128, F], f32)
                    for t in range(KH * KW):
                        dy, dx = t // KW, t % KW
                        off = q0 + dy * WP + dx - XCH_LO[ck]
                        nc.tensor.matmul(
                            ps[:],
                            xcs[ck][:, off : off + 128],
                            wtc[t // 3][:, t % 3, :],
                            start=(t == 0),
                            stop=(t == 8),
                        )
                    # bias add on DVE (in PSUM), relu+copyout on ScalarE
                    nc.vector.tensor_add(ps[:], ps[:], bt[:])
                    ot = opool.tile([128, F], f32)
                    nc.scalar.activation(
                        ot[:], ps[:], mybir.ActivationFunctionType.Relu
                    )
                    nc.sync.dma_start(y[b, m], ot[:])
    nc.compile()
    return nc


def _prep_inputs(X, W, bias, Werr, Berr):
    bf16 = ml_dtypes.bfloat16
    X, W, bias, Werr, Berr = (
        np.asarray(a) for a in (X, W, bias, Werr, Berr)
    )
    # per-sample perturbed kernels, laid out [B, Cin, tap, F]
    memW = (W[None] * Werr).transpose(0, 3, 1, 2, 4).reshape(B, Cin, KH * KW, F)
    memW = np.ascontiguousarray(memW, dtype=bf16)
    # padded image, flattened with one lead zero so all tap offsets are >= 0
    Xpad = np.zeros((B, Cin, HP, WP), dtype=bf16)
    Xpad[:, :, 1 : H + 1, 1 : Wd + 1] = X.transpose(0, 3, 1, 2)
    Xp = np.zeros((B, Cin, XLEN), dtype=bf16)
    Xp[:, :, 1 : 1 + HP * WP] = Xpad.reshape(B, Cin, HP * WP)
    # bias broadcast across the 128 spatial partitions of an output tile
    mbias = (bias[None] * Berr).astype(np.float32)  # [B, F]
    mbias = np.ascontiguousarray(
        np.broadcast_to(mbias[:, None, :], (B, 128, F))
    )
    return Xp, memW, mbias


def _postprocess(y_cores):
    # y per core: [BPC, MT, 128, F] over padded positions (64 x 66);
    # drop the junk columns xp=0 and xp=65
    out = np.concatenate(y_cores, axis=0)  # [B, MT, 128, F]
    out = out.reshape(B, H, WP, F)[:, :, 1 : Wd + 1, :]
    return np.ascontiguousarray(out)


def kernel(X, W, bias, Werr, Berr):
    global _compiled
    from concourse.bass_utils import run_bass_kernel_spmd

    if _compiled is None:
        _compiled = _build_bass()
    nc = _compiled

    Xp, memW, mbias = _prep_inputs(X, W, bias, Werr, Berr)
    in_maps = [
        {
            "xp": Xp[c * BPC : (c + 1) * BPC],
            "wm": memW[c * BPC : (c + 1) * BPC],
            "mb": mbias[c * BPC : (c + 1) * BPC],
        }
        for c in range(NCORES)
    ]
    res = run_bass_kernel_spmd(nc, in_maps, core_ids=list(range(NCORES)))
    return _postprocess([r["y"] for r in res.results])


# revision 20
# speedup vs baseline: 1.0106x; 1.0106x over previous
"""A-Connect conv kernel for TRN2, data-parallel over batch on 8 NeuronCores.

Computation (per sample b):
    Z[b] = conv2d(X[b], W * Werr[b], SAME) + bias * Berr[b]; out = relu(Z)

Mapping: batch 32 -> 4 samples per core. Per sample the conv is 9
shifted matmuls accumulated in PSUM. The output is computed in the
zero-padded spatial geometry (64 rows x 66 cols = 4224 = 33 tiles of
128 positions): in that flattened geometry the stationary operand for
tap (dy, dx) is a single stride-1 run of the padded input at offset
q0 + dy*66 + dx, which satisfies the BIR rule that matmul operand APs
have one free dimension. The two junk columns (xp = 0, 65) are sliced
away on the host. PSUM/output tiles are [spatial, F], matching NHWC so
stores are contiguous; the per-sample bias is added on the vector
engine in PSUM and relu+copyout runs on the scalar engine. A burst of
dependency-free warmup matmuls at kernel start releases the PE HAM
clock gate while the first input DMAs are in flight. Inputs are
pre-transposed/padded on host and fed in bf16 (measured rel err vs the
fp32 reference: 2.3e-3).
"""

import numpy as np
import ml_dtypes

B, H, Wd, Cin, F, KH, KW = 32, 64, 64, 128, 256, 3, 3
NCORES = 8
BPC = B // NCORES  # samples per core
HP, WP = H + 2, Wd + 2  # zero-padded spatial
NQ = H * WP  # padded output positions per sample: 64*66 = 4224
MT = NQ // 128  # 33 M-tiles of 128 padded positions
XLEN = 4368  # 1 lead zero + 66*66 flat padded image + tail zeros
# X arrives in overlapping chunks (small first) so matmuls can start as
# soon as a few hundred KB have landed. Chunk k serves M-tiles
# [XCH_MT[k], XCH_MT[k+1]) and covers flat [XCH_LO[k], XCH_HI[k]):
# a tile m reads offsets [128m, 128m+134+128).
XCH_MT = [0, 2, 8, 16, 24, MT]
XCH_LO = [0, 256, 1024, 2048, 3072]
XCH_HI = [390, 1158, 2182, 3206, XLEN]

_compiled = None  # cached Bass program so repeated kernel() calls reuse it


def _build_bass():
    from concourse import bacc, tile, mybir

    nc = bacc.Bacc("TRN2", target_bir_lowering=False, debug=False)
    bf16 = mybir.dt.bfloat16
    f32 = mybir.dt.float32

    xp = nc.dram_tensor("xp", [BPC, Cin, XLEN], bf16, kind="ExternalInput")
    wm = nc.dram_tensor("wm", [BPC, Cin, KH * KW, F], bf16, kind="ExternalInput")
    mb = nc.dram_tensor("mb", [BPC, 128, F], f32, kind="ExternalInput")
    y = nc.dram_tensor("y", [BPC, MT, 128, F], f32, kind="ExternalOutput")

    with tile.TileContext(nc) as tc:
        with (
            tc.tile_pool(name="xpool", bufs=2) as xpool,
            tc.tile_pool(name="wpool", bufs=2) as wpool,
            tc.tile_pool(name="bpool", bufs=2) as bpool,
            tc.tile_pool(name="opool", bufs=8) as opool,
            tc.tile_pool(name="cpool", bufs=1) as cpool,
            tc.tile_pool(name="pspool", bufs=5, space="PSUM") as pspool,
            tc.tile_pool(name="pslast", bufs=2, space="PSUM") as pslpool,
            tc.tile_pool(name="wupool", bufs=1, space="PSUM") as wupool,
        ):
            # PE warmup: ~5us of dependency-free matmuls so the HAM clock
            # gate is released (K=8/8) by the time the first input DMA lands
            # ~6.4us of cold N=512 matmuls: the HAM clock-gate window is
            # free-running, so worst case needs ~6.8us of sustained PE
            # activity before release; shorter warmups make the real
            # stream start cold (measured +2us)
            wu_in = cpool.tile([128, 512], bf16)
            nc.vector.memset(wu_in[:], 0.0)
            wu_ps = wupool.tile([128, 512], f32)
            for i in range(20):
                nc.tensor.matmul(
                    wu_ps[:],
                    wu_in[:, :128],
                    wu_in[:],
                    start=(i == 0),
                    stop=(i == 19),
                )
            for b in range(BPC):
                # input DMAs ride the scalar engine's HWDGE ring so their
                # issue cost doesn't contend with output DMAs on nc.sync;
                # weights and X stream in small-first chunks to cut the
                # lead-in before the first matmul
                wtc = []
                xcs = []
                for j in range(KH):
                    w_ = wpool.tile([Cin, KW, F], bf16, tag=f"wt{j}")
                    nc.scalar.dma_start(w_[:], wm[b, :, 3 * j : 3 * j + 3, :])
                    wtc.append(w_)
                    if j < len(XCH_LO):
                        lo, hi = XCH_LO[j], XCH_HI[j]
                        xc = xpool.tile([Cin, hi - lo], bf16, tag=f"xc{j}")
                        nc.scalar.dma_start(xc[:], xp[b, :, lo:hi])
                        xcs.append(xc)
                for j in range(KH, len(XCH_LO)):
                    lo, hi = XCH_LO[j], XCH_HI[j]
                    xc = xpool.tile([Cin, hi - lo], bf16, tag=f"xc{j}")
                    nc.scalar.dma_start(xc[:], xp[b, :, lo:hi])
                    xcs.append(xc)
                bt = bpool.tile([128, F], f32)
                nc.scalar.dma_start(bt[:], mb[b])
                ck = 0
                for m in range(MT):
                    q0 = m * 128
                    while m >= XCH_MT[ck + 1]:
                        ck += 1
                    if b == BPC - 1 and m == MT - 1:
                        # the kernel's very last group: split into two N=128
                        # half-groups (separate PSUM banks) so the first
                        # half's ADD/RELU/DMA chain overlaps the second
                        # half's matmuls, shortening the kernel tail
                        for h in range(2):
                            ph = pslpool.tile([128, F // 2], f32)
                            for t in range(KH * KW):
                                dy, dx = t // KW, t % KW
                                off = q0 + dy * WP + dx - XCH_LO[ck]
                                nc.tensor.matmul(
                                    ph[:],
                                    xcs[ck][:, off : off + 128],
                                    wtc[t // 3][:, t % 3, h * 128 : h * 128 + 128],
                                    start=(t == 0),
                                    stop=(t == 8),
                                )
                            nc.vector.tensor_add(
                                ph[:], ph[:], bt[:, h * 128 : h * 128 + 128]
                            )
                            oh = opool.tile([128, F // 2], f32, tag="otlast")
                            nc.scalar.activation(
                                oh[:], ph[:], mybir.ActivationFunctionType.Relu
                            )
                            nc.sync.dma_start(
                                y[b, m, :, h * 128 : h * 128 + 128], oh[:]
                            )
                        continue
                    ps = pspool.tile([128, F], f32)
                    for t in range(KH * KW):
                        dy, dx = t // KW, t % KW
                        off = q0 + dy * WP + dx - XCH_LO[ck]
                        nc.tensor.matmul(
                            ps[:],
                            xcs[ck][:, off : off + 128],
                            wtc[t // 3][:, t % 3, :],
                            start=(t == 0),
                            stop=(t == 8),
                        )
                    # bias add on DVE (in PSUM), relu+copyout on ScalarE
                    nc.vector.tensor_add(ps[:], ps[:], bt[:])
                    ot = opool.tile([128, F], f32)
                    nc.scalar.activation(
                        ot[:], ps[:], mybir.ActivationFunctionType.Relu
                    )
                    nc.sync.dma_start(y[b, m], ot[:])
    nc.compile()
    return nc


def _prep_inputs(X, W, bias, Werr, Berr):
    bf16 = ml_dtypes.bfloat16
    X, W, bias, Werr, Berr = (
        np.asarray(a) for a in (X, W, bias, Werr, Berr)
    )
    # per-sample perturbed kernels, laid out [B, Cin, tap, F]
    memW = (W[None] * Werr).transpose(0, 3, 1, 2, 4).reshape(B, Cin, KH * KW, F)
    memW = np.ascontiguousarray(memW, dtype=bf16)
    # padded image, flattened with one lead zero so all tap offsets are >= 0
    Xpad = np.zeros((B, Cin, HP, WP), dtype=bf16)
    Xpad[:, :, 1 : H + 1, 1 : Wd + 1] = X.transpose(0, 3, 1, 2)
    Xp = np.zeros((B, Cin, XLEN), dtype=bf16)
    Xp[:, :, 1 : 1 + HP * WP] = Xpad.reshape(B, Cin, HP * WP)
    # bias broadcast across the 128 spatial partitions of an output tile
    mbias = (bias[None] * Berr).astype(np.float32)  # [B, F]
    mbias = np.ascontiguousarray(
        np.broadcast_to(mbias[:, None, :], (B, 128, F))
    )
    return Xp, memW, mbias


def _postprocess(y_cores):
    # y per core: [BPC, MT, 128, F] over padded positions (64 x 66);
    # drop the junk columns xp=0 and xp=65
    out = np.concatenate(y_cores, axis=0)  # [B, MT, 128, F]
    out = out.reshape(B, H, WP, F)[:, :, 1 : Wd + 1, :]
    return np.ascontiguousarray(out)


def kernel(X, W, bias, Werr, Berr):
    global _compiled
    from concourse.bass_utils import run_bass_kernel_spmd

    if _compiled is None:
        _compiled = _build_bass()
    nc = _compiled

    Xp, memW, mbias = _prep_inputs(X, W, bias, Werr, Berr)
    in_maps = [
        {
            "xp": Xp[c * BPC : (c + 1) * BPC],
            "wm": memW[c * BPC : (c + 1) * BPC],
            "mb": mbias[c * BPC : (c + 1) * BPC],
        }
        for c in range(NCORES)
    ]
    res = run_bass_kernel_spmd(nc, in_maps, core_ids=list(range(NCORES)))
    return _postprocess([r["y"] for r in res.results])
